# revision 1
# baseline (speedup 1.0000x reference)
"""Trainium2 Bass kernel for nn_CCM: per-pixel complex 3x3 conv mask.

Math (per batch element b, sharded 1 batch element per NeuronCore):
  y[t,f] = sum_{c=0..26} m[c,t,f] * (w_{k(c)} * X)[t+i(c)-2, f+j(c)-1]
where c = 9*k + 3*i + j, w_k = v[0,k] + 1j*v[1,k] (cube roots of unity),
X = xr + 1j*xi, zero padded (causal in t: 2 top; symmetric in f: 1,1).

Layout: t = 8*p + tau, partitions p in [0,125), (tau, f) in the free dim,
so every tap shift is a free-dim offset read of padded "U" planes
U_k = w_k * X stored as [125, 10 tau-slots, 259 f-cols] (slots tau=-2..7).
"""

import sys
import numpy as np

sys.path.insert(0, "/opt/trn_rl_repo")

B = 8
C = 27
T = 1000
F = 257
TP = 125          # partitions
TAU = 8           # t = 8*p + tau
NS = 10           # tau slots in U planes: tau in [-2, 8)
FP = 259          # padded f width: f in [-1, 258)
SQ3H = float(np.sqrt(3.0) / 2.0)

_CACHE = {}


def _emit(ctx, tc, m_ap, x_ap, id_ap, y_ap):
    import concourse.mybir as mybir

    nc = tc.nc
    f32 = mybir.dt.float32
    FCS = [(0, 128), (128, 128), (256, 1)]  # f chunks for transposes

    const = ctx.enter_context(tc.tile_pool(name="const", bufs=1))
    planes = ctx.enter_context(tc.tile_pool(name="planes", bufs=1))
    mpool = ctx.enter_context(tc.tile_pool(name="mtiles", bufs=3))
    work = ctx.enter_context(tc.tile_pool(name="work", bufs=3))
    psum = ctx.enter_context(tc.tile_pool(name="psum", bufs=3, space="PSUM"))

    ident = const.tile([128, 128], f32, tag="ident")
    nc.sync.dma_start(ident[:], id_ap)

    # ---- load x in natural layout: [f, (tt, comp)] with tt = t + 2 (2 zero rows)
    xns = []
    for (f0, fw) in FCS:
        xn = const.tile([fw, (T + 2) * 2], f32, tag=f"xn{f0}")
        nc.vector.memset(xn[:, 0:4], 0.0)
        nc.sync.dma_start(
            xn[:, 4:], x_ap[f0:f0 + fw].rearrange("f t c -> f (t c)")
        )
        xns.append(xn)

    # ---- transpose x into blocked padded planes xr, xi: [TP, NS, FP]
    xq = []
    for q in range(2):
        p = planes.tile([TP, NS, FP], f32, tag=f"xq{q}")
        nc.vector.memset(p[:], 0.0)
        xq.append(p)
    for q in range(2):
        for ts in range(NS):  # slot ts corresponds to tau = ts - 2; tt = 8p + ts
            for ci, (f0, fw) in enumerate(FCS):
                pt = psum.tile([TP, 128], f32, tag="tp")
                xn3 = xns[ci].rearrange("f (t c) -> f t c", c=2)
                nc.tensor.transpose(
                    pt[0:TP, 0:fw],
                    xn3[0:fw, ts:ts + TAU * (TP - 1) + 1:TAU, q],
                    ident[0:fw, 0:fw],
                )
                nc.scalar.copy(xq[q][:, ts, 1 + f0:1 + f0 + fw], pt[0:TP, 0:fw])

    # ---- U planes: U_k = w_k * (xr + i xi), w_k = exp(+-2pi i/3), w_0 = 1
    mult = mybir.AluOpType.mult
    add = mybir.AluOpType.add
    sub = mybir.AluOpType.subtract
    t1 = planes.tile([TP, NS, FP], f32, tag="t1")
    t2 = planes.tile([TP, NS, FP], f32, tag="t2")
    ur1 = planes.tile([TP, NS, FP], f32, tag="ur1")
    ui1 = planes.tile([TP, NS, FP], f32, tag="ui1")
    ur2 = planes.tile([TP, NS, FP], f32, tag="ur2")
    ui2 = planes.tile([TP, NS, FP], f32, tag="ui2")
    nc.vector.tensor_scalar_mul(t1[:], xq[1][:], SQ3H)  # xi * s
    nc.vector.tensor_scalar_mul(t2[:], xq[0][:], SQ3H)  # xr * s
    nc.vector.scalar_tensor_tensor(ur1[:], xq[0][:], -0.5, t1[:], op0=mult, op1=sub)
    nc.vector.scalar_tensor_tensor(ui1[:], xq[1][:], -0.5, t2[:], op0=mult, op1=add)
    nc.vector.scalar_tensor_tensor(ur2[:], xq[0][:], -0.5, t1[:], op0=mult, op1=add)
    nc.vector.scalar_tensor_tensor(ui2[:], xq[1][:], -0.5, t2[:], op0=mult, op1=sub)
    U = [(xq[0], xq[1]), (ur1, ui1), (ur2, ui2)]

    # ---- tap loop: acc += m_c * U_k[shifted]
    acc_r = planes.tile([TP, TAU, F], f32, tag="accr")
    acc_i = planes.tile([TP, TAU, F], f32, tag="acci")
    for c in range(C):
        kk, n = divmod(c, 9)
        i, j = divmod(n, 3)
        dt, df = i - 2, j - 1
        mt = mpool.tile([TP, TAU * F], f32, tag="mt")
        nc.sync.dma_start(mt[:], m_ap[c].rearrange("(p t) f -> p (t f)", p=TP))
        m3 = mt.rearrange("p (t f) -> p t f", f=F)
        ur, ui = U[kk]
        urs = ur[:, dt + 2:dt + 2 + TAU, df + 1:df + 1 + F]
        uis = ui[:, dt + 2:dt + 2 + TAU, df + 1:df + 1 + F]
        if c == 0:
            nc.vector.tensor_mul(acc_r[:], m3[:], urs)
            nc.vector.tensor_mul(acc_i[:], m3[:], uis)
        else:
            pr = work.tile([TP, TAU, F], f32, tag="prod")
            nc.vector.tensor_mul(pr[:], m3[:], urs)
            nc.vector.tensor_add(acc_r[:], acc_r[:], pr[:])
            pi = work.tile([TP, TAU, F], f32, tag="prod")
            nc.vector.tensor_mul(pi[:], m3[:], uis)
            nc.vector.tensor_add(acc_i[:], acc_i[:], pi[:])

    # ---- transpose back to [f, (t, comp)] and store
    for ci, (f0, fw) in enumerate(FCS):
        yo = const.tile([fw, T * 2], f32, tag=f"yo{f0}")
        yv = yo.rearrange("f (t c) -> f t c", c=2)
        for comp, acc in ((0, acc_r), (1, acc_i)):
            for ts in range(TAU):
                pt = psum.tile([128, TP], f32, tag="tp2")
                nc.tensor.transpose(
                    pt[0:fw, 0:TP], acc[:, ts, f0:f0 + fw], ident[0:TP, 0:TP]
                )
                nc.scalar.copy(
                    yv[0:fw, ts:ts + TAU * (TP - 1) + 1:TAU, comp], pt[0:fw, 0:TP]
                )
        nc.sync.dma_start(y_ap[f0:f0 + fw].rearrange("f t c -> f (t c)"), yo[:])


def _build():
    if "nc" in _CACHE:
        return _CACHE["nc"]
    from contextlib import ExitStack
    from concourse import bacc, mybir
    import concourse.tile as tile

    f32 = mybir.dt.float32
    nc = bacc.Bacc("TRN2", target_bir_lowering=False, debug=False, num_devices=B)
    m_d = nc.dram_tensor("m", (C, T, F), f32, kind="ExternalInput")
    x_d = nc.dram_tensor("x", (F, T, 2), f32, kind="ExternalInput")
    id_d = nc.dram_tensor("ident", (128, 128), f32, kind="ExternalInput")
    y_d = nc.dram_tensor("y", (F, T, 2), f32, kind="ExternalOutput")

    with tile.TileContext(nc) as tc:
        with ExitStack() as ctx:
            _emit(ctx, tc, m_d.ap(), x_d.ap(), id_d.ap(), y_d.ap())
    nc.compile()
    _CACHE["nc"] = nc
    return nc


def _in_maps(m, x):
    ident = np.eye(128, dtype=np.float32)
    return [
        {"m": np.ascontiguousarray(m[b]), "x": np.ascontiguousarray(x[b]),
         "ident": ident}
        for b in range(B)
    ]


def kernel(m, x, v, _trace=False):
    from concourse import bass_utils

    m = np.asarray(m, dtype=np.float32)
    x = np.asarray(x, dtype=np.float32)
    nc = _build()
    res = bass_utils.run_bass_kernel_spmd(
        nc, _in_maps(m, x), core_ids=list(range(B)), trace=_trace
    )
    kernel.last_results = res
    y = np.stack([res.results[b]["y"] for b in range(B)], axis=0)
    return y



# revision 10
# speedup vs baseline: 1.0914x; 1.0914x over previous
"""Trainium2 Bass kernel for nn_CCM: per-pixel complex 3x3 conv mask.

Math (per batch element b, sharded 1 batch element per NeuronCore):
  y[t,f] = sum_{c=0..26} m[c,t,f] * (w_{k(c)} * X)[t+i(c)-2, f+j(c)-1]
where c = 9*k + 3*i + j, w_k = v[0,k] + 1j*v[1,k] (cube roots of unity),
X = xr + 1j*xi, zero padded (causal in t: 2 top; symmetric in f: 1,1).

Layout: t = 8*p + tau, partitions p in [0,125), (tau, f) in the free dim,
so every tap shift is a free-dim offset read of padded "U" planes
U_k = w_k * X stored as [125, 10 tau-slots, 259 f-cols] (slots tau=-2..7).

Perf structure (v2): fp16 datapath so DVE TensorTensor runs in 2x_1p mode
(2 elem/cycle/lane). 21 taps accumulate on DVE in fp16 (ACT converts their
m planes f32->fp16); 6 taps accumulate on Pool (gpsimd) in f32 straight
from the DMA'd m (Pool is dtype-agnostic in speed). Transposes run in fp16
on PE. m DMAs are interleaved so Pool-owned taps arrive early.
"""

import sys
import numpy as np

sys.path.insert(0, "/opt/trn_rl_repo")

B = 8
C = 27
T = 1000
F = 257
TP = 125          # partitions
TAU = 8           # t = 8*p + tau
NS = 10           # tau slots in U planes: tau in [-2, 8)
FP = 259          # padded f width: f in [-1, 258)
SQ3H = float(np.sqrt(3.0) / 2.0)

POOL_TAPS = (18, 19, 20, 21, 22, 23)   # k=2 group, accumulated on Pool in f32
DMA_ORDER = [18, 0, 19, 1, 20, 2, 21, 3] + list(range(4, 18)) + [22, 23, 24, 25, 26]

_CACHE = {}


def _emit(ctx, tc, m_ap, x_ap, id_ap, y_ap):
    import concourse.mybir as mybir

    nc = tc.nc
    f32 = mybir.dt.float32
    f16 = mybir.dt.float16
    FCS = [(0, 128), (128, 128), (256, 1)]  # f chunks for transposes

    const = ctx.enter_context(tc.tile_pool(name="const", bufs=1))
    planes = ctx.enter_context(tc.tile_pool(name="planes", bufs=1))
    mpool = ctx.enter_context(tc.tile_pool(name="mtiles", bufs=4))
    mppool = ctx.enter_context(tc.tile_pool(name="mptiles", bufs=4))
    m16pool = ctx.enter_context(tc.tile_pool(name="m16tiles", bufs=3))
    work = ctx.enter_context(tc.tile_pool(name="work", bufs=3))
    xstage = ctx.enter_context(tc.tile_pool(name="xstage", bufs=1))
    pwork = ctx.enter_context(tc.tile_pool(name="pwork", bufs=2))
    psum = ctx.enter_context(tc.tile_pool(name="psum", bufs=3, space="PSUM"))

    ident = const.tile([128, 128], f32, tag="ident")
    nc.sync.dma_start(ident[:], id_ap)
    ident16 = const.tile([128, 128], f16, tag="ident16")
    nc.scalar.copy(ident16[:], ident[:])

    # ---- load x in natural layout: [f, (tt, comp)] with tt = t + 2 (2 zero rows)
    xn16s = []
    for (f0, fw) in FCS:
        xn = xstage.tile([128, (T + 2) * 2], f32, tag="xn", name="xn")[0:fw]
        nc.vector.memset(xn[:, 0:4], 0.0)
        nc.sync.dma_start(
            xn[:, 4:], x_ap[f0:f0 + fw].rearrange("f t c -> f (t c)")
        )
        xn16 = const.tile([fw, (T + 2) * 2], f16, tag=f"xn16_{f0}")
        nc.scalar.copy(xn16[:], xn[:])
        xn16s.append(xn16)

    # ---- m DMAs (interleaved order so Pool taps arrive early)
    mtiles = {}
    for c in DMA_ORDER:
        pool = mppool if c in POOL_TAPS else mpool
        mt = pool.tile([TP, TAU * F], f32, tag="mtp" if c in POOL_TAPS else "mt")
        nc.sync.dma_start(mt[:], m_ap[c].rearrange("(p t) f -> p (t f)", p=TP))
        mtiles[c] = mt

    # ---- transpose x into blocked padded planes xr, xi: [TP, NS, FP] fp16
    xq = []
    for q in range(2):
        p = planes.tile([TP, NS, FP], f16, tag=f"xq{q}")
        # only pad cols 0 and 258 need zeroing; transposes fill 1..257
        nc.vector.memset(p[:, :, 0:FP:FP - 1], 0.0)
        xq.append(p)
    for q in range(2):
        for ts in range(NS):  # slot ts corresponds to tau = ts - 2; tt = 8p + ts
            for ci, (f0, fw) in enumerate(FCS):
                pt = psum.tile([TP, 128], f16, tag="tp")
                xn3 = xn16s[ci].rearrange("f (t c) -> f t c", c=2)
                nc.tensor.transpose(
                    pt[0:TP, 0:fw],
                    xn3[0:fw, ts:ts + TAU * (TP - 1) + 1:TAU, q],
                    ident16[0:fw, 0:fw],
                )
                nc.scalar.copy(xq[q][:, ts, 1 + f0:1 + f0 + fw], pt[0:TP, 0:fw])

    # ---- U planes: U_k = w_k * (xr + i xi), w_k = exp(+-2pi i/3), w_0 = 1
    mult = mybir.AluOpType.mult
    add = mybir.AluOpType.add
    sub = mybir.AluOpType.subtract
    t1 = planes.tile([TP, NS, FP], f16, tag="t1")
    t2 = planes.tile([TP, NS, FP], f16, tag="t2")
    ur1 = planes.tile([TP, NS, FP], f16, tag="ur1")
    ui1 = planes.tile([TP, NS, FP], f16, tag="ui1")
    ur2 = planes.tile([TP, NS, FP], f16, tag="ur2")
    ui2 = planes.tile([TP, NS, FP], f16, tag="ui2")
    # all on DVE: tensor_scalar gets 4x fp16 mode, tensor_tensor gets 2x;
    # Pool lacks scalar_tensor_tensor so build U from TS/TT only.
    nc.vector.tensor_scalar_mul(t1[:], xq[1][:], SQ3H)    # s*xi
    nc.vector.tensor_scalar_mul(t2[:], xq[0][:], SQ3H)    # s*xr
    nc.vector.tensor_scalar_mul(ur1[:], xq[0][:], -0.5)   # -xr/2 (tmp)
    nc.vector.tensor_scalar_mul(ui1[:], xq[1][:], -0.5)   # -xi/2 (tmp)
    nc.vector.tensor_sub(ur2[:], ur1[:], t1[:])           # -xr/2 - s*xi ... k=2 real
    nc.vector.tensor_add(ur1[:], ur1[:], t1[:])           # -xr/2 + s*xi ... k=1 real? (sign check below)
    nc.vector.tensor_add(ui2[:], ui1[:], t2[:])           # -xi/2 + s*xr
    nc.vector.tensor_sub(ui1[:], ui1[:], t2[:])           # -xi/2 - s*xr
    # w_1 = -1/2 + i*s: U_1 = w_1*(xr+i*xi) -> re = -xr/2 - s*xi, im = -xi/2 + s*xr
    # w_2 = -1/2 - i*s: U_2 = w_2*(xr+i*xi) -> re = -xr/2 + s*xi, im = -xi/2 - s*xr
    U = [(xq[0], xq[1]), (ur2, ui2), (ur1, ui1)]

    # ---- tap loops
    # DVE taps: fp16 chain (m converted on ACT); Pool taps: f32 chain.
    acc_r = planes.tile([TP, TAU, F], f16, tag="accr")
    acc_i = planes.tile([TP, TAU, F], f16, tag="acci")
    pacc_r = planes.tile([TP, TAU, F], f16, tag="paccr")
    pacc_i = planes.tile([TP, TAU, F], f16, tag="pacci")

    first_dve = True
    first_pool = True
    for c in range(C):
        kk, n = divmod(c, 9)
        i, j = divmod(n, 3)
        dt, df = i - 2, j - 1
        ur, ui = U[kk]
        urs = ur[:, dt + 2:dt + 2 + TAU, df + 1:df + 1 + F]
        uis = ui[:, dt + 2:dt + 2 + TAU, df + 1:df + 1 + F]
        mt = mtiles[c]
        m3f = mt.rearrange("p (t f) -> p t f", f=F)
        if c in POOL_TAPS:
            if first_pool:
                nc.gpsimd.tensor_mul(pacc_r[:], m3f[:], urs)
                nc.gpsimd.tensor_mul(pacc_i[:], m3f[:], uis)
                first_pool = False
            else:
                pr = pwork.tile([TP, TAU, F], f16, tag="pprod")
                nc.gpsimd.tensor_mul(pr[:], m3f[:], urs)
                nc.gpsimd.tensor_add(pacc_r[:], pacc_r[:], pr[:])
                pi = pwork.tile([TP, TAU, F], f16, tag="pprod")
                nc.gpsimd.tensor_mul(pi[:], m3f[:], uis)
                nc.gpsimd.tensor_add(pacc_i[:], pacc_i[:], pi[:])
        else:
            m16 = m16pool.tile([TP, TAU * F], f16, tag="m16")
            nc.scalar.copy(m16[:], mt[:])
            m3 = m16.rearrange("p (t f) -> p t f", f=F)
            if first_dve:
                nc.vector.tensor_mul(acc_r[:], m3[:], urs)
                nc.vector.tensor_mul(acc_i[:], m3[:], uis)
                first_dve = False
            else:
                pr = work.tile([TP, TAU, F], f16, tag="prod")
                nc.vector.tensor_mul(pr[:], m3[:], urs)
                nc.vector.tensor_add(acc_r[:], acc_r[:], pr[:])
                pi = work.tile([TP, TAU, F], f16, tag="prod")
                nc.vector.tensor_mul(pi[:], m3[:], uis)
                nc.vector.tensor_add(acc_i[:], acc_i[:], pi[:])

    # ---- combine DVE + Pool partial accumulators in place (fp16, 2x mode)
    nc.vector.tensor_add(acc_r[:], acc_r[:], pacc_r[:])
    nc.vector.tensor_add(acc_i[:], acc_i[:], pacc_i[:])

    # ---- transpose back to [f, (t, comp)] and store
    for ci, (f0, fw) in enumerate(FCS):
        yo = const.tile([fw, T * 2], f32, tag=f"yo{f0}")
        yv = yo.rearrange("f (t c) -> f t c", c=2)
        for comp, acc in ((0, acc_r), (1, acc_i)):
            for ts in range(TAU):
                pt = psum.tile([128, TP], f16, tag="tp2")
                nc.tensor.transpose(
                    pt[0:fw, 0:TP], acc[:, ts, f0:f0 + fw], ident16[0:TP, 0:TP]
                )
                nc.scalar.copy(
                    yv[0:fw, ts:ts + TAU * (TP - 1) + 1:TAU, comp], pt[0:fw, 0:TP]
                )
        nc.sync.dma_start(y_ap[f0:f0 + fw].rearrange("f t c -> f (t c)"), yo[:])


def _build():
    if "nc" in _CACHE:
        return _CACHE["nc"]
    from contextlib import ExitStack
    from concourse import bacc, mybir
    import concourse.tile as tile

    f32 = mybir.dt.float32
    nc = bacc.Bacc("TRN2", target_bir_lowering=False, debug=False, num_devices=B)
    m_d = nc.dram_tensor("m", (C, T, F), f32, kind="ExternalInput")
    x_d = nc.dram_tensor("x", (F, T, 2), f32, kind="ExternalInput")
    id_d = nc.dram_tensor("ident", (128, 128), f32, kind="ExternalInput")
    y_d = nc.dram_tensor("y", (F, T, 2), f32, kind="ExternalOutput")

    with tile.TileContext(nc) as tc:
        with ExitStack() as ctx:
            _emit(ctx, tc, m_d.ap(), x_d.ap(), id_d.ap(), y_d.ap())
    nc.compile()
    _CACHE["nc"] = nc
    return nc


def _in_maps(m, x):
    ident = np.eye(128, dtype=np.float32)
    return [
        {"m": np.ascontiguousarray(m[b]), "x": np.ascontiguousarray(x[b]),
         "ident": ident}
        for b in range(B)
    ]


def kernel(m, x, v, _trace=False):
    from concourse import bass_utils

    m = np.asarray(m, dtype=np.float32)
    x = np.asarray(x, dtype=np.float32)
    nc = _build()
    res = bass_utils.run_bass_kernel_spmd(
        nc, _in_maps(m, x), core_ids=list(range(B)), trace=_trace
    )
    kernel.last_results = res
    y = np.stack([res.results[b]["y"] for b in range(B)], axis=0)
    return y


# revision 14
# speedup vs baseline: 1.1475x; 1.0514x over previous
"""Trainium2 Bass kernel for nn_CCM: per-pixel complex 3x3 conv mask.

Math (per batch element b, sharded 1 batch element per NeuronCore):
  y[t,f] = sum_{c=0..26} m[c,t,f] * (w_{k(c)} * X)[t+i(c)-2, f+j(c)-1]
where c = 9*k + 3*i + j, w_k = v[0,k] + 1j*v[1,k] (cube roots of unity),
X = xr + 1j*xi, zero padded (causal in t: 2 top; symmetric in f: 1,1).

Layout: t = 8*p + tau, partitions p in [0,125), (tau, f) in the free dim.
U planes are stored as complex PAIRS [TP, NS, 2, FP] fp16 so one DVE
TensorTensor handles both real and imag of a tap (m broadcast stride-0
across the comp dim). All fp16 tensors use even-element row strides
(FP=260, FW=258) so every innermost run is 4-byte aligned -> DVE 2x_1p
perf mode engages (2 elem/cycle/lane). Center taps (df=0) would start at
an odd column, so k=0 center taps read a pre-shifted copy B0; the k=1
center taps go to Pool (alignment-agnostic); c=25 stays 1x on DVE.
PSUM->SBUF copies are merged 8-slots-at-a-time to cut ACT overhead.
"""

import sys
import numpy as np

sys.path.insert(0, "/opt/trn_rl_repo")

B = 8
C = 27
T = 1000
F = 257
TP = 125          # partitions
TAU = 8           # t = 8*p + tau
NS = 10           # tau slots in U planes: tau in [-2, 8)
FP = 260          # padded f width (f in [-1, 258); col 259 = alignment pad)
FW = 258          # row width of m16/pr/acc tiles (col 257 = alignment pad)
SQ3H = float(np.sqrt(3.0) / 2.0)

POOL_TAPS = (10, 13, 16, 19, 22)   # on Pool: k=1 centers + 2 k=2 centers
# DVE compute order: k0 (B0-aligned centers), then k1, k2 leftovers
DVE_TAPS = [0, 1, 2, 3, 4, 5, 6, 7, 8, 9, 11, 12, 14, 15, 17, 18, 20, 21, 23, 24, 25, 26]
B0_TAPS = (1, 4, 7)                # k=0 centers read shifted plane B0
# DMA order: feeds DVE's k0 burst early, pool taps just in time
DMA_ORDER = [0, 10, 1, 2, 3, 4, 5, 13, 6, 7, 8, 16, 9, 19, 11, 12, 14, 15, 22,
             17, 18, 20, 21, 23, 24, 25, 26]

_CACHE = {}


def _emit(ctx, tc, m_ap, x_ap, id_ap, y_ap):
    import concourse.mybir as mybir

    nc = tc.nc
    f32 = mybir.dt.float32
    f16 = mybir.dt.float16
    FCS = [(0, 128), (128, 128), (256, 1)]  # f chunks for transposes

    const = ctx.enter_context(tc.tile_pool(name="const", bufs=1))
    planes = ctx.enter_context(tc.tile_pool(name="planes", bufs=1))
    mpool = ctx.enter_context(tc.tile_pool(name="mtiles", bufs=3))
    m16d = ctx.enter_context(tc.tile_pool(name="m16d", bufs=3))
    m16p = ctx.enter_context(tc.tile_pool(name="m16p", bufs=4))
    work = ctx.enter_context(tc.tile_pool(name="work", bufs=2))
    pwork = ctx.enter_context(tc.tile_pool(name="pwork", bufs=1))
    xstage = ctx.enter_context(tc.tile_pool(name="xstage", bufs=1))
    psum = ctx.enter_context(tc.tile_pool(name="psum", bufs=2, space="PSUM"))

    ident = const.tile([128, 128], f32, tag="ident")
    nc.sync.dma_start(ident[:], id_ap)
    ident16 = const.tile([128, 128], f16, tag="ident16")
    nc.scalar.copy(ident16[:], ident[:])

    # ---- U plane pair tiles; u0 memset for pad cols (0, 258, 259)
    u0 = planes.tile([TP, NS, 2, FP], f16, tag="u0")
    u1 = planes.tile([TP, NS, 2, FP], f16, tag="u1")
    u2 = planes.tile([TP, NS, 2, FP], f16, tag="u2")
    b0 = planes.tile([TP, NS, 2, FP], f16, tag="b0")
    nc.vector.memset(u0[:], 0.0)

    # ---- load x in natural layout [f, (tt, comp)] (tt = t + 2), convert fp16
    xn16s = []
    for (f0, fw) in FCS:
        xn = xstage.tile([128, (T + 2) * 2], f32, tag="xn", name="xn")[0:fw]
        nc.vector.memset(xn[:, 0:4], 0.0)
        nc.sync.dma_start(
            xn[:, 4:], x_ap[f0:f0 + fw].rearrange("f t c -> f (t c)")
        )
        xn16 = const.tile([fw, (T + 2) * 2], f16, tag=f"xn16_{f0}")
        nc.scalar.copy(xn16[:], xn[:])
        xn16s.append(xn16)

    # ---- m DMAs
    mtiles = {}
    for c in DMA_ORDER:
        mt = mpool.tile([TP, TAU * F], f32, tag="mt", name="mt")
        nc.sync.dma_start(mt[:], m_ap[c].rearrange("(p t) f -> p (t f)", p=TP))
        mtiles[c] = mt

    # ---- transpose x into u0 [TP, NS, 2, FP]; merged 8-slot PSUM copies
    for q in range(2):
        for ci, (f0, fw) in enumerate(FCS):
            xn3 = xn16s[ci].rearrange("f (t c) -> f t c", c=2)
            tp8 = psum.tile([TP, TAU, 128], f16, tag="tp8", name="tp8")
            for ts in range(TAU):
                nc.tensor.transpose(
                    tp8[0:TP, ts, 0:fw],
                    xn3[0:fw, ts:ts + TAU * (TP - 1) + 1:TAU, q],
                    ident16[0:fw, 0:fw],
                )
            nc.scalar.copy(u0[:, 0:TAU, q, 1 + f0:1 + f0 + fw], tp8[:, :, 0:fw])
            tp2 = psum.tile([TP, 2, 128], f16, tag="tp2", name="tp2")
            for ts in range(TAU, NS):
                nc.tensor.transpose(
                    tp2[0:TP, ts - TAU, 0:fw],
                    xn3[0:fw, ts:ts + TAU * (TP - 1) + 1:TAU, q],
                    ident16[0:fw, 0:fw],
                )
            nc.scalar.copy(u0[:, TAU:NS, q, 1 + f0:1 + f0 + fw], tp2[:, :, 0:fw])

    # ---- U planes: U_1 = w_1*X, U_2 = w_2*X (w = -1/2 +- i*sqrt(3)/2)
    x_r = u0[:, :, 0, :]
    x_i = u0[:, :, 1, :]
    t1 = planes.tile([TP, NS, FP], f16, tag="t1")
    t2 = planes.tile([TP, NS, FP], f16, tag="t2")
    nc.vector.tensor_scalar_mul(t1[:], x_i, SQ3H)            # s*xi
    nc.vector.tensor_scalar_mul(t2[:], x_r, SQ3H)            # s*xr
    nc.vector.tensor_scalar_mul(u1[:, :, 0, :], x_r, -0.5)   # -xr/2
    nc.vector.tensor_scalar_mul(u1[:, :, 1, :], x_i, -0.5)   # -xi/2
    nc.vector.tensor_sub(u2[:, :, 0, :], u1[:, :, 0, :], t1[:])
    nc.vector.tensor_add(u1[:, :, 0, :], u1[:, :, 0, :], t1[:])
    nc.vector.tensor_add(u2[:, :, 1, :], u1[:, :, 1, :], t2[:])
    nc.vector.tensor_sub(u1[:, :, 1, :], u1[:, :, 1, :], t2[:])
    # After the in-place updates:
    #   u2.re = -xr/2 - s*xi (U_1 real),  u2.im = -xi/2 + s*xr (U_1 imag)
    #   u1.re = -xr/2 + s*xi (U_2 real),  u1.im = -xi/2 - s*xr (U_2 imag)
    # so the k->plane map is U[1] = u2, U[2] = u1.
    U = [u0, u2, u1]

    # ---- B0: u0 shifted left one column so df=0 taps start 4B-aligned
    nc.scalar.copy(b0[:, :, :, 0:FW], u0[:, :, :, 1:FP - 1])

    # ---- tap loops (paired complex ops; m broadcast across comp dim)
    acc = planes.tile([TP, TAU, 2, FW], f16, tag="acc")
    pacc = planes.tile([TP, TAU, 2, FW], f16, tag="pacc")

    def u_slice(c):
        kk, n = divmod(c, 9)
        i, j = divmod(n, 3)
        dt, df = i - 2, j - 1
        if c in B0_TAPS:
            return b0[:, dt + 2:dt + 2 + TAU, :, 0:F]
        return U[kk][:, dt + 2:dt + 2 + TAU, :, df + 1:df + 1 + F]

    def m_bcast(m16):
        return m16[:, :, 0:F].unsqueeze(2).broadcast_to((TP, TAU, 2, F))

    # Pool chain (k=1 centers + 19, 22): all fp16
    firstp = True
    for c in POOL_TAPS:
        m16 = m16p.tile([TP, TAU, FW], f16, tag="m16p", name="m16p")
        nc.scalar.copy(m16[:, :, 0:F], mtiles[c].rearrange("p (t f) -> p t f", f=F))
        if firstp:
            nc.gpsimd.tensor_mul(pacc[:, :, :, 0:F], m_bcast(m16), u_slice(c))
            firstp = False
        else:
            pr = pwork.tile([TP, TAU, 2, FW], f16, tag="pprod", name="pprod")
            nc.gpsimd.tensor_mul(pr[:, :, :, 0:F], m_bcast(m16), u_slice(c))
            nc.gpsimd.tensor_add(pacc[:, :, :, 0:F], pacc[:, :, :, 0:F],
                                 pr[:, :, :, 0:F])

    # DVE chain
    firstd = True
    for c in DVE_TAPS:
        m16 = m16d.tile([TP, TAU, FW], f16, tag="m16d", name="m16d")
        nc.scalar.copy(m16[:, :, 0:F], mtiles[c].rearrange("p (t f) -> p t f", f=F))
        if firstd:
            nc.vector.tensor_mul(acc[:, :, :, 0:F], m_bcast(m16), u_slice(c))
            firstd = False
        else:
            pr = work.tile([TP, TAU, 2, FW], f16, tag="prod", name="prod")
            nc.vector.tensor_mul(pr[:, :, :, 0:F], m_bcast(m16), u_slice(c))
            nc.vector.tensor_add(acc[:, :, :, 0:F], acc[:, :, :, 0:F],
                                 pr[:, :, :, 0:F])

    # ---- combine DVE + Pool accumulators in place
    nc.vector.tensor_add(acc[:, :, :, 0:F], acc[:, :, :, 0:F], pacc[:, :, :, 0:F])

    # ---- transpose back to [f, (t, comp)]; merged 8-tau PSUM copies
    for ci, (f0, fw) in enumerate(FCS):
        yo = const.tile([fw, T * 2], f32, tag=f"yo{f0}")
        yv = yo.rearrange("f (p s c) -> f s p c", s=TAU, c=2)
        for comp in range(2):
            tpo = psum.tile([128, TAU, 126], f16, tag="tpo", name="tpo")
            for ts in range(TAU):
                nc.tensor.transpose(
                    tpo[0:fw, ts, 0:TP], acc[:, ts, comp, f0:f0 + fw],
                    ident16[0:TP, 0:TP],
                )
            nc.scalar.copy(yv[0:fw, :, :, comp], tpo[0:fw, :, 0:TP])
        nc.sync.dma_start(y_ap[f0:f0 + fw].rearrange("f t c -> f (t c)"), yo[:])


def _build():
    if "nc" in _CACHE:
        return _CACHE["nc"]
    from contextlib import ExitStack
    from concourse import bacc, mybir
    import concourse.tile as tile

    f32 = mybir.dt.float32
    nc = bacc.Bacc("TRN2", target_bir_lowering=False, debug=False, num_devices=B)
    m_d = nc.dram_tensor("m", (C, T, F), f32, kind="ExternalInput")
    x_d = nc.dram_tensor("x", (F, T, 2), f32, kind="ExternalInput")
    id_d = nc.dram_tensor("ident", (128, 128), f32, kind="ExternalInput")
    y_d = nc.dram_tensor("y", (F, T, 2), f32, kind="ExternalOutput")

    with tile.TileContext(nc) as tc:
        with ExitStack() as ctx:
            _emit(ctx, tc, m_d.ap(), x_d.ap(), id_d.ap(), y_d.ap())
    nc.compile()
    _CACHE["nc"] = nc
    return nc


def _in_maps(m, x):
    ident = np.eye(128, dtype=np.float32)
    return [
        {"m": np.ascontiguousarray(m[b]), "x": np.ascontiguousarray(x[b]),
         "ident": ident}
        for b in range(B)
    ]


def kernel(m, x, v, _trace=False):
    from concourse import bass_utils

    m = np.asarray(m, dtype=np.float32)
    x = np.asarray(x, dtype=np.float32)
    nc = _build()
    res = bass_utils.run_bass_kernel_spmd(
        nc, _in_maps(m, x), core_ids=list(range(B)), trace=_trace
    )
    kernel.last_results = res
    y = np.stack([res.results[b]["y"] for b in range(B)], axis=0)
    return y


# revision 15
# speedup vs baseline: 1.1709x; 1.0204x over previous
"""Trainium2 Bass kernel for nn_CCM: per-pixel complex 3x3 conv mask.

Math (per batch element b, sharded 1 batch element per NeuronCore):
  y[t,f] = sum_{c=0..26} m[c,t,f] * (w_{k(c)} * X)[t+i(c)-2, f+j(c)-1]
where c = 9*k + 3*i + j, w_k = v[0,k] + 1j*v[1,k] (cube roots of unity),
X = xr + 1j*xi, zero padded (causal in t: 2 top; symmetric in f: 1,1).

Layout: t = 8*p + tau, partitions p in [0,125), (tau, f) in the free dim.
U planes are stored as complex PAIRS [TP, NS, 2, FP] fp16 so one DVE
TensorTensor handles both real and imag of a tap (m broadcast stride-0
across the comp dim). All fp16 tensors use even-element row strides
(FP=260, FW=258) so every innermost run is 4-byte aligned -> DVE 2x_1p
perf mode engages (2 elem/cycle/lane). Center taps (df=0) would start at
an odd column, so k=0 center taps read a pre-shifted copy B0; the k=1
center taps go to Pool (alignment-agnostic); c=25 stays 1x on DVE.
PSUM->SBUF copies are merged 8-slots-at-a-time to cut ACT overhead.
"""

import sys
import numpy as np

sys.path.insert(0, "/opt/trn_rl_repo")

B = 8
C = 27
T = 1000
F = 257
TP = 125          # partitions
TAU = 8           # t = 8*p + tau
NS = 10           # tau slots in U planes: tau in [-2, 8)
FP = 260          # padded f width (f in [-1, 258); col 259 = alignment pad)
FW = 258          # row width of m16/pr/acc tiles (col 257 = alignment pad)
SQ3H = float(np.sqrt(3.0) / 2.0)

POOL_TAPS = (10, 13, 16, 19, 22)   # on Pool: k=1 centers + 2 k=2 centers
# DVE compute order: k0 (B0-aligned centers), then k1, k2 leftovers
DVE_TAPS = [0, 1, 2, 3, 4, 5, 6, 7, 8, 9, 11, 12, 14, 15, 17, 18, 20, 21, 23, 24, 25, 26]
B0_TAPS = (1, 4, 7)                # k=0 centers read shifted plane B0
# DMA order: feeds DVE's k0 burst early, pool taps just in time
DMA_ORDER = [0, 10, 1, 2, 3, 4, 5, 13, 6, 7, 8, 16, 9, 19, 11, 12, 14, 15, 22,
             17, 18, 20, 21, 23, 24, 25, 26]

_CACHE = {}


def _emit(ctx, tc, m_ap, x_ap, id_ap, y_ap):
    import concourse.mybir as mybir

    nc = tc.nc
    f32 = mybir.dt.float32
    f16 = mybir.dt.float16
    FCS = [(0, 128), (128, 128), (256, 1)]  # f chunks for transposes

    const = ctx.enter_context(tc.tile_pool(name="const", bufs=1))
    planes = ctx.enter_context(tc.tile_pool(name="planes", bufs=1))
    mpool = ctx.enter_context(tc.tile_pool(name="mtiles", bufs=3))
    m16d = ctx.enter_context(tc.tile_pool(name="m16d", bufs=3))
    m16p = ctx.enter_context(tc.tile_pool(name="m16p", bufs=3))
    work = ctx.enter_context(tc.tile_pool(name="work", bufs=2))
    pwork = ctx.enter_context(tc.tile_pool(name="pwork", bufs=1))
    xstage = ctx.enter_context(tc.tile_pool(name="xstage", bufs=2))
    psum = ctx.enter_context(tc.tile_pool(name="psum", bufs=2, space="PSUM"))

    ident = const.tile([128, 128], f32, tag="ident")
    nc.sync.dma_start(ident[:], id_ap)
    ident16 = const.tile([128, 128], f16, tag="ident16")
    nc.scalar.copy(ident16[:], ident[:])

    # ---- U plane pair tiles; u0 memset for pad cols (0, 258, 259)
    u0 = planes.tile([TP, NS, 2, FP], f16, tag="u0")
    u1 = planes.tile([TP, NS, 2, FP], f16, tag="u1")
    u2 = planes.tile([TP, NS, 2, FP], f16, tag="u2")
    b0 = planes.tile([TP, NS, 2, FP], f16, tag="b0")
    nc.gpsimd.memset(u0[:], 0.0)

    # ---- load x in natural layout [f, (tt, comp)] (tt = t + 2), convert fp16
    xn16s = []
    for (f0, fw) in FCS:
        xn = xstage.tile([128, (T + 2) * 2], f32, tag="xn", name="xn")[0:fw]
        nc.vector.memset(xn[:, 0:4], 0.0)
        nc.sync.dma_start(
            xn[:, 4:], x_ap[f0:f0 + fw].rearrange("f t c -> f (t c)")
        )
        xn16 = const.tile([fw, (T + 2) * 2], f16, tag=f"xn16_{f0}")
        nc.scalar.copy(xn16[:], xn[:])
        xn16s.append(xn16)

    # ---- m DMAs
    mtiles = {}
    for c in DMA_ORDER:
        mt = mpool.tile([TP, TAU * F], f32, tag="mt", name="mt")
        nc.sync.dma_start(mt[:], m_ap[c].rearrange("(p t) f -> p (t f)", p=TP))
        mtiles[c] = mt

    # ---- transpose x into u0 [TP, NS, 2, FP]; merged 8-slot PSUM copies
    for q in range(2):
        for ci, (f0, fw) in enumerate(FCS):
            xn3 = xn16s[ci].rearrange("f (t c) -> f t c", c=2)
            tp8 = psum.tile([TP, TAU, 128], f16, tag="tp8", name="tp8")
            for ts in range(TAU):
                nc.tensor.transpose(
                    tp8[0:TP, ts, 0:fw],
                    xn3[0:fw, ts:ts + TAU * (TP - 1) + 1:TAU, q],
                    ident16[0:fw, 0:fw],
                )
            nc.scalar.copy(u0[:, 0:TAU, q, 1 + f0:1 + f0 + fw], tp8[:, :, 0:fw])
            tp2 = psum.tile([TP, 2, 128], f16, tag="tp2", name="tp2")
            for ts in range(TAU, NS):
                nc.tensor.transpose(
                    tp2[0:TP, ts - TAU, 0:fw],
                    xn3[0:fw, ts:ts + TAU * (TP - 1) + 1:TAU, q],
                    ident16[0:fw, 0:fw],
                )
            nc.scalar.copy(u0[:, TAU:NS, q, 1 + f0:1 + f0 + fw], tp2[:, :, 0:fw])

    # ---- U planes: U_1 = w_1*X, U_2 = w_2*X (w = -1/2 +- i*sqrt(3)/2)
    x_r = u0[:, :, 0, :]
    x_i = u0[:, :, 1, :]
    t1 = planes.tile([TP, NS, FP], f16, tag="t1")
    t2 = planes.tile([TP, NS, FP], f16, tag="t2")
    nc.vector.tensor_scalar_mul(t1[:], x_i, SQ3H)            # s*xi
    nc.vector.tensor_scalar_mul(t2[:], x_r, SQ3H)            # s*xr
    nc.vector.tensor_scalar_mul(u1[:, :, 0, :], x_r, -0.5)   # -xr/2
    nc.vector.tensor_scalar_mul(u1[:, :, 1, :], x_i, -0.5)   # -xi/2
    nc.vector.tensor_sub(u2[:, :, 0, :], u1[:, :, 0, :], t1[:])
    nc.vector.tensor_add(u1[:, :, 0, :], u1[:, :, 0, :], t1[:])
    nc.vector.tensor_add(u2[:, :, 1, :], u1[:, :, 1, :], t2[:])
    nc.vector.tensor_sub(u1[:, :, 1, :], u1[:, :, 1, :], t2[:])
    # After the in-place updates:
    #   u2.re = -xr/2 - s*xi (U_1 real),  u2.im = -xi/2 + s*xr (U_1 imag)
    #   u1.re = -xr/2 + s*xi (U_2 real),  u1.im = -xi/2 - s*xr (U_2 imag)
    # so the k->plane map is U[1] = u2, U[2] = u1.
    U = [u0, u2, u1]

    # ---- B0: u0 shifted left one column so df=0 taps start 4B-aligned
    nc.gpsimd.tensor_copy(b0[:, :, :, 0:FW], u0[:, :, :, 1:FP - 1])

    # ---- tap loops (paired complex ops; m broadcast across comp dim)
    acc = planes.tile([TP, TAU, 2, FW], f16, tag="acc")
    pacc = planes.tile([TP, TAU, 2, FW], f16, tag="pacc")

    def u_slice(c):
        kk, n = divmod(c, 9)
        i, j = divmod(n, 3)
        dt, df = i - 2, j - 1
        if c in B0_TAPS:
            return b0[:, dt + 2:dt + 2 + TAU, :, 0:F]
        return U[kk][:, dt + 2:dt + 2 + TAU, :, df + 1:df + 1 + F]

    def m_bcast(m16):
        return m16[:, :, 0:F].unsqueeze(2).broadcast_to((TP, TAU, 2, F))

    # Pool chain (k=1 centers + 19, 22): all fp16
    firstp = True
    for c in POOL_TAPS:
        m16 = m16p.tile([TP, TAU, FW], f16, tag="m16p", name="m16p")
        nc.scalar.copy(m16[:, :, 0:F], mtiles[c].rearrange("p (t f) -> p t f", f=F))
        if firstp:
            nc.gpsimd.tensor_mul(pacc[:, :, :, 0:F], m_bcast(m16), u_slice(c))
            firstp = False
        else:
            pr = pwork.tile([TP, TAU, 2, FW], f16, tag="pprod", name="pprod")
            nc.gpsimd.tensor_mul(pr[:, :, :, 0:F], m_bcast(m16), u_slice(c))
            nc.gpsimd.tensor_add(pacc[:, :, :, 0:F], pacc[:, :, :, 0:F],
                                 pr[:, :, :, 0:F])

    # DVE chain: stride-0 broadcast kills DVE throughput, so each tap does
    # two plain 3D muls (real/imag, both 2x-aligned) + one paired add.
    firstd = True
    for c in DVE_TAPS:
        m16 = m16d.tile([TP, TAU, FW], f16, tag="m16d", name="m16d")
        nc.scalar.copy(m16[:, :, 0:F], mtiles[c].rearrange("p (t f) -> p t f", f=F))
        us = u_slice(c)
        dst = acc if firstd else work.tile([TP, TAU, 2, FW], f16, tag="prod",
                                           name="prod")
        nc.vector.tensor_mul(dst[:, :, 0, 0:F], m16[:, :, 0:F], us[:, :, 0, :])
        nc.vector.tensor_mul(dst[:, :, 1, 0:F], m16[:, :, 0:F], us[:, :, 1, :])
        if firstd:
            firstd = False
        else:
            nc.vector.tensor_add(acc[:, :, :, 0:F], acc[:, :, :, 0:F],
                                 dst[:, :, :, 0:F])

    # ---- combine DVE + Pool accumulators in place
    nc.vector.tensor_add(acc[:, :, :, 0:F], acc[:, :, :, 0:F], pacc[:, :, :, 0:F])

    # ---- transpose back to [f, (t, comp)]; merged 8-tau PSUM copies
    for ci, (f0, fw) in enumerate(FCS):
        yo = const.tile([fw, T * 2], f32, tag=f"yo{f0}")
        yv = yo.rearrange("f (p s c) -> f s p c", s=TAU, c=2)
        for comp in range(2):
            tpo = psum.tile([128, TAU, 126], f16, tag="tpo", name="tpo")
            for ts in range(TAU):
                nc.tensor.transpose(
                    tpo[0:fw, ts, 0:TP], acc[:, ts, comp, f0:f0 + fw],
                    ident16[0:TP, 0:TP],
                )
            nc.scalar.copy(yv[0:fw, :, :, comp], tpo[0:fw, :, 0:TP])
        nc.sync.dma_start(y_ap[f0:f0 + fw].rearrange("f t c -> f (t c)"), yo[:])


def _build():
    if "nc" in _CACHE:
        return _CACHE["nc"]
    from contextlib import ExitStack
    from concourse import bacc, mybir
    import concourse.tile as tile

    f32 = mybir.dt.float32
    nc = bacc.Bacc("TRN2", target_bir_lowering=False, debug=False, num_devices=B)
    m_d = nc.dram_tensor("m", (C, T, F), f32, kind="ExternalInput")
    x_d = nc.dram_tensor("x", (F, T, 2), f32, kind="ExternalInput")
    id_d = nc.dram_tensor("ident", (128, 128), f32, kind="ExternalInput")
    y_d = nc.dram_tensor("y", (F, T, 2), f32, kind="ExternalOutput")

    with tile.TileContext(nc) as tc:
        with ExitStack() as ctx:
            _emit(ctx, tc, m_d.ap(), x_d.ap(), id_d.ap(), y_d.ap())
    nc.compile()
    _CACHE["nc"] = nc
    return nc


def _in_maps(m, x):
    ident = np.eye(128, dtype=np.float32)
    return [
        {"m": np.ascontiguousarray(m[b]), "x": np.ascontiguousarray(x[b]),
         "ident": ident}
        for b in range(B)
    ]


def kernel(m, x, v, _trace=False):
    from concourse import bass_utils

    m = np.asarray(m, dtype=np.float32)
    x = np.asarray(x, dtype=np.float32)
    nc = _build()
    res = bass_utils.run_bass_kernel_spmd(
        nc, _in_maps(m, x), core_ids=list(range(B)), trace=_trace
    )
    kernel.last_results = res
    y = np.stack([res.results[b]["y"] for b in range(B)], axis=0)
    return y
